# revision 1
# baseline (speedup 1.0000x reference)
"""Trainium2 Bass kernel for nn_DSnetwork (gnn_message_passing).

Reference computation (S=131072 subgraphs, G=4096 graphs, N=2M nodes, D=128):
  h_sub  = segment_mean(h_node, subgraph_batch, S)            # [S,128]
  2x DS layers:
    x1 = h_sub @ W + b
    x2 = segment_mean(h_sub, subgraph_idx_batch, G) @ Ws + bs
    h_sub = elu(x1 + x2[subgraph_idx_batch])
  h_graph = segment_mean(h_sub, subgraph_idx_batch, G)
  out = relu(h_graph @ Wf1 + bf1) @ Wf2 + bf2                 # [G,10]

Distribution: data-parallel over contiguous graph ranges (512 graphs per
core, 8 cores).  Indices are sorted, so each core owns contiguous slices
of subgraphs and nodes.  Segment sums run on TensorE as one-hot matmuls
(one-hots built on VectorE from host-precomputed relative ids); the
graph->subgraph broadcast is a transposed-one-hot matmul accumulated
directly into the x1 PSUM tile.  Matmuls are bf16 with fp32 PSUM
accumulation; mean scaling is exact fp32 on ScalarE.

Host-side work is pure index preprocessing and data staging: sharding,
padded placement of subgraphs/nodes into static tiles, relative one-hot
ids, 1/count scale vectors, and dtype casts.
"""

from dataclasses import dataclass

import ml_dtypes
import numpy as np

BF16 = ml_dtypes.bfloat16
P = 128


@dataclass(frozen=True)
class Cfg:
    D: int = 128          # node feature dim
    C: int = 128          # hidden dim
    NCORES: int = 8
    G_SH: int = 512       # graphs per core
    NGC: int = 4          # graph chunks of 128 graphs per core
    T2: int = 34          # seg tiles (128 segs) per graph chunk, padded
    W: int = 128          # phase-1 one-hot window (segs per psum chunk)
    T1: int = 17          # node tile slots (128 nodes) per W-seg chunk
    SWATH: int = 8        # seg tiles per elu swath (ragged tail ok)

    @property
    def NS(self):         # seg tiles per core (padded axis)
        return self.NGC * self.T2

    @property
    def SMAXP(self):      # padded segs per core
        return self.NS * P

    @property
    def NCH(self):        # phase-1 chunks per core
        return self.SMAXP // self.W

    @property
    def NSLOT(self):      # node tile slots per core
        return self.NCH * self.T1


FULL = Cfg()

# ---------------------------------------------------------------------------
# host-side planner: shard + metadata layout
# ---------------------------------------------------------------------------


def _plan_core(cfg, core, h_node_bf16, sb, sib, seg_cnt, g_cnt):
    g0 = core * cfg.G_SH
    W = cfg.W

    hp = np.zeros((cfg.NSLOT * P, cfg.D), dtype=BF16)
    rel = np.full((P, cfg.NSLOT, 2), -1.0, dtype=BF16)
    invs = np.zeros((P, cfg.NS), dtype=np.float32)
    rel2 = np.full((P, cfg.NS, 2), -1.0, dtype=BF16)
    invg = np.zeros((P, cfg.NGC), dtype=np.float32)

    # chunk-local graph id per padded seg (-1 pad; fits bf16 exactly)
    gid_pad = np.full(cfg.SMAXP, -1.0, dtype=np.float64)

    for gc in range(cfg.NGC):
        glo = g0 + gc * P
        ghi = glo + P
        a = int(np.searchsorted(sib, glo))
        b = int(np.searchsorted(sib, ghi))
        nseg = b - a
        assert nseg <= cfg.T2 * P, f"T2 too small: {nseg} > {cfg.T2 * P}"
        base_tile = gc * cfg.T2          # first seg tile of this graph chunk

        gl = (sib[a:b] - glo).astype(np.int64)          # in [0,128)
        pad_pos = base_tile * P
        gid_pad[pad_pos:pad_pos + nseg] = gl

        r2 = np.full(cfg.T2 * P, -1.0, dtype=BF16)
        r2[:nseg] = gl.astype(BF16)
        rel2[:, base_tile:base_tile + cfg.T2, :] = r2.reshape(cfg.T2, P).T[:, :, None]

        ivs = np.zeros(cfg.T2 * P, dtype=np.float32)
        ivs[:nseg] = 1.0 / np.maximum(seg_cnt[a:b], 1).astype(np.float32)
        invs[:, base_tile:base_tile + cfg.T2] = ivs.reshape(cfg.T2, P).T

        invg[:, gc] = 1.0 / np.maximum(g_cnt[glo:ghi], 1).astype(np.float32)

        # node packing: phase-1 chunks of W padded segs
        seg_starts = np.searchsorted(sb, np.arange(a, b + 1))
        nch_per_gc = cfg.T2 * P // W
        for cc in range(nch_per_gc):
            c = (base_tile * P) // W + cc            # global chunk index
            slo = cc * W
            shi = min(slo + W, nseg)
            if slo >= nseg:
                continue
            nlo = int(seg_starts[slo])
            nhi = int(seg_starts[shi])
            nn = nhi - nlo
            assert nn <= cfg.T1 * P, f"T1 too small: {nn} > {cfg.T1 * P}"
            if nn == 0:
                continue
            dst = c * cfg.T1 * P
            hp[dst:dst + nn] = h_node_bf16[nlo:nhi]
            rr = (sb[nlo:nhi] - (a + slo)).astype(BF16)
            rfull = np.full(cfg.T1 * P, -1.0, dtype=BF16)
            rfull[:nn] = rr
            rel[:, c * cfg.T1:(c + 1) * cfg.T1, :] = rfull.reshape(cfg.T1, P).T[:, :, None]

    # device layout: hp_dram[c, p, t*D+d] = node row (c*T1*P + t*P + p)
    hp_dev = np.ascontiguousarray(
        hp.reshape(cfg.NCH, cfg.T1, P, cfg.D).transpose(0, 2, 1, 3)
    ).reshape(cfg.NCH, P, cfg.T1 * cfg.D)

    gidb = np.broadcast_to(gid_pad.astype(BF16), (P, cfg.SMAXP)).copy()

    return {
        "hp": hp_dev,
        "rel": rel,
        "invs": invs,
        "rel2": rel2,
        "invg": invg,
        "gidb": gidb,
    }


def plan(cfg, h_node, sb, sib):
    sb = np.asarray(sb).astype(np.int64)
    sib = np.asarray(sib).astype(np.int64)
    S = sib.shape[0]
    G = cfg.NCORES * cfg.G_SH
    seg_cnt = np.bincount(sb, minlength=S)
    g_cnt = np.bincount(sib, minlength=G)
    h_bf16 = np.asarray(h_node).astype(BF16)
    return [
        _plan_core(cfg, c, h_bf16, sb, sib, seg_cnt, g_cnt)
        for c in range(cfg.NCORES)
    ]


# ---------------------------------------------------------------------------
# bass program
# ---------------------------------------------------------------------------


def build_bass(cfg):
    import concourse.mybir as mybir
    import concourse.tile as tile
    from concourse import bacc

    f32 = mybir.dt.float32
    bf16 = mybir.dt.bfloat16
    AF = mybir.ActivationFunctionType
    OP = mybir.AluOpType
    D, C, W = cfg.D, cfg.C, cfg.W
    HALVES = P // W          # psum chunks per seg tile
    TPW = cfg.T2 * P         # padded segs per graph chunk

    nc = bacc.Bacc("TRN2", target_bir_lowering=False, debug=False)

    def din(name, shape, dt=f32):
        return nc.dram_tensor(name, shape, dt, kind="ExternalInput").ap()

    hp_d = din("hp", [cfg.NCH, P, cfg.T1 * D], bf16)
    rel_d = din("rel", [P, cfg.NSLOT, 2], bf16)
    invs_d = din("invs", [P, cfg.NS])
    rel2_d = din("rel2", [P, cfg.NS, 2], bf16)
    invg_d = din("invg", [P, cfg.NGC])
    gidb_d = din("gidb", [P, cfg.SMAXP], bf16)
    iota_d = din("iota", [P, P], bf16)
    iotag_d = din("iotag", [P, cfg.NGC])
    ident_d = din("ident", [P, P], bf16)

    w_d = {}
    for l in range(2):
        w_d[f"W{l}"] = din(f"W{l}", [D, C])
        w_d[f"Ws{l}"] = din(f"Ws{l}", [D, C])
        w_d[f"b{l}"] = din(f"b{l}", [C])
        w_d[f"bs{l}"] = din(f"bs{l}", [C])
    w_d["Wf1"] = din("Wf1", [C, 2 * C])
    w_d["bf1"] = din("bf1", [2 * C])
    w_d["Wf2"] = din("Wf2", [2 * C, 10])
    w_d["bf2"] = din("bf2", [10])

    out_d = nc.dram_tensor("out", [10, cfg.G_SH], f32, kind="ExternalOutput").ap()

    with tile.TileContext(nc) as tc:
        with (
            tc.tile_pool(name="persist", bufs=1) as pp,
            tc.tile_pool(name="stream", bufs=2) as sp,
            tc.tile_pool(name="small", bufs=2) as mp,
            tc.tile_pool(name="psum_acc", bufs=4, space="PSUM") as pacc,
            tc.tile_pool(name="psum_tr", bufs=2, space="PSUM") as ptr,
            tc.tile_pool(name="psum_wide", bufs=2, space="PSUM") as pwide,
        ):
            # ---- constants / weights to SBUF -------------------------------
            def load(ap_dram, shape, dt):
                t = pp.tile(shape, dt, tag=f"ld_{ap_dram.tensor.name}")
                nc.sync.dma_start(t[:], ap_dram)
                return t

            iota = load(iota_d, [P, P], bf16)
            iotag = load(iotag_d, [P, cfg.NGC], f32)
            ident = load(ident_d, [P, P], bf16)
            rel = load(rel_d, [P, cfg.NSLOT, 2], bf16)
            invs = load(invs_d, [P, cfg.NS], f32)
            rel2 = load(rel2_d, [P, cfg.NS, 2], bf16)
            invg = load(invg_d, [P, cfg.NGC], f32)

            def cast_bf16(name, tf, shape):
                tb = pp.tile(shape, bf16, tag=f"bf_{name}")
                nc.vector.tensor_copy(tb[:], tf[:])
                return tb

            Wl, Ws, bsum = [], [], []
            for l in range(2):
                Wl.append(cast_bf16(
                    f"W{l}", load(w_d[f"W{l}"], [D, C], f32), [D, C]))
                Ws.append(cast_bf16(
                    f"Ws{l}", load(w_d[f"Ws{l}"], [D, C], f32), [D, C]))
                b_t = load(w_d[f"b{l}"].unsqueeze(1), [P, 1], f32)
                bs_t = load(w_d[f"bs{l}"].unsqueeze(1), [P, 1], f32)
                s = pp.tile([P, 1], f32, tag=f"bsum{l}")
                nc.vector.tensor_tensor(s[:], b_t[:], bs_t[:], op=OP.add)
                bsum.append(s)
            Wf1 = cast_bf16("Wf1", load(w_d["Wf1"], [C, 2 * C], f32),
                            [C, 2 * C])
            Wf2 = cast_bf16(
                "Wf2",
                load(w_d["Wf2"].rearrange("(h p) t -> p h t", h=2),
                     [P, 2, 10], f32),
                [P, 2, 10])
            bf1 = load(w_d["bf1"].rearrange("(h p) -> p h", h=2), [P, 2], f32)
            bf2_t = pp.tile([P, 1], f32, tag="ld_bf2")
            nc.sync.dma_start(bf2_t[:10, :], w_d["bf2"].unsqueeze(1))

            # persistent activations: per graph chunk [seg_p, (t2, d)]
            hs_a = [pp.tile([P, cfg.T2, D], bf16, tag=f"hsa{gc}", name=f"hsa{gc}")
                    for gc in range(cfg.NGC)]
            hs_b = [pp.tile([P, cfg.T2, D], bf16, tag=f"hsb{gc}", name=f"hsb{gc}")
                    for gc in range(cfg.NGC)]
            ohg = [pp.tile([P, cfg.T2, P], bf16, tag=f"ohg{gc}", name=f"ohg{gc}")
                   for gc in range(cfg.NGC)]

            # ---- graph-level one-hots (built once, reused) -----------------
            for gc in range(cfg.NGC):
                r2b = rel2[:, gc * cfg.T2:(gc + 1) * cfg.T2, :] \
                    .unsqueeze(2).to_broadcast([P, cfg.T2, P // 2, 2])
                iob = iota[:].rearrange("p (a b) -> p a b", b=2) \
                    .unsqueeze(1).to_broadcast([P, cfg.T2, P // 2, 2])
                ohv = ohg[gc][:].rearrange("p a (b c) -> p a b c", c=2)
                nc.vector.tensor_tensor(ohv, r2b, iob, op=OP.is_equal)

            # ---- phase 1: node -> subgraph mean ----------------------------
            for cch in range(cfg.NCH):
                k, hh = cch // HALVES, cch % HALVES     # seg tile, half
                gc, t2 = k // cfg.T2, k % cfg.T2
                hpt = sp.tile([P, cfg.T1 * D], bf16, tag="hp", bufs=3)
                nc.sync.dma_start(hpt[:], hp_d[cch])
                oh = sp.tile([P, cfg.T1, W], bf16, tag="oh", bufs=3)
                rb = rel[:, cch * cfg.T1:(cch + 1) * cfg.T1, :] \
                    .unsqueeze(2).to_broadcast([P, cfg.T1, W // 2, 2])
                iob = iota[:, :W].rearrange("p (a b) -> p a b", b=2) \
                    .unsqueeze(1).to_broadcast([P, cfg.T1, W // 2, 2])
                ohv = oh[:].rearrange("p a (b c) -> p a b c", c=2)
                nc.vector.tensor_tensor(ohv, rb, iob, op=OP.is_equal)
                ps = pacc.tile([P, D], f32, tag="acc")
                for t in range(cfg.T1):
                    nc.tensor.matmul(
                        ps[:W, :], lhsT=oh[:, t, :],
                        rhs=hpt[:, t * D:(t + 1) * D],
                        start=(t == 0), stop=(t == cfg.T1 - 1))
                nc.scalar.activation(
                    hs_a[gc][hh * W:(hh + 1) * W, t2, :], ps[:W, :], AF.Copy,
                    scale=invs[hh * W:(hh + 1) * W, k:k + 1])

            # ---- DS layers -------------------------------------------------
            hs_in, hs_out = hs_a, hs_b
            for l in range(2):
                # graph means -> transposed [d, g] table
                gmT = mp.tile([P, cfg.NGC * P], bf16, tag="gmT")
                for gc in range(cfg.NGC):
                    psg = pacc.tile([P, D], f32, tag="acc")
                    for t2 in range(cfg.T2):
                        nc.tensor.matmul(
                            psg[:], lhsT=ohg[gc][:, t2, :],
                            rhs=hs_in[gc][:, t2, :],
                            start=(t2 == 0), stop=(t2 == cfg.T2 - 1))
                    gm = mp.tile([P, D], bf16, tag="gm")
                    nc.scalar.activation(gm[:], psg[:], AF.Copy,
                                         scale=invg[:, gc:gc + 1])
                    ptt = ptr.tile([P, P], bf16, tag="tr")
                    nc.tensor.transpose(ptt[:], gm[:], ident[:])
                    nc.scalar.activation(gmT[:, gc * P:(gc + 1) * P], ptt[:], AF.Copy)

                # x2 = gmean @ Ws + (b + bs), row-major in SBUF
                x2ps = pwide.tile([P, cfg.NGC * P], f32, tag="wide")
                nc.tensor.matmul(x2ps[:], lhsT=Ws[l][:], rhs=gmT[:],
                                 start=True, stop=True)
                x2T = mp.tile([P, cfg.NGC * P], bf16, tag="x2T")
                nc.scalar.activation(x2T[:], x2ps[:], AF.Identity,
                                     bias=bsum[l][:])
                x2rm = mp.tile([P, cfg.NGC, C], bf16, tag="x2rm")
                for gc in range(cfg.NGC):
                    ptt = ptr.tile([P, P], bf16, tag="tr")
                    nc.tensor.transpose(ptt[:], x2T[:, gc * P:(gc + 1) * P],
                                        ident[:])
                    nc.scalar.activation(x2rm[:, gc, :], ptt[:], AF.Copy)

                for gc in range(cfg.NGC):
                    # transposed graph one-hot [g, seg] for the x2 broadcast
                    gsl = sp.tile([P, TPW], bf16, tag="gsl")
                    nc.sync.dma_start(
                        gsl[:], gidb_d[:, gc * TPW:(gc + 1) * TPW])
                    ohgT = gsl
                    nc.vector.tensor_scalar(
                        ohgT[:], gsl[:], iotag[:, gc:gc + 1], None,
                        op0=OP.is_equal)
                    for s0 in range(0, cfg.T2, cfg.SWATH):
                        sl = min(cfg.SWATH, cfg.T2 - s0)
                        comb = mp.tile([P, cfg.SWATH, C], f32, tag="comb")
                        for j in range(sl):
                            t2 = s0 + j
                            ptt = ptr.tile([P, P], bf16, tag="tr")
                            nc.tensor.transpose(ptt[:], hs_in[gc][:, t2, :],
                                                ident[:])
                            hT = mp.tile([P, P], bf16, tag="hT")
                            nc.vector.tensor_copy(hT[:], ptt[:])
                            x1p = pacc.tile([P, C], f32, tag="acc")
                            nc.tensor.matmul(x1p[:], lhsT=hT[:], rhs=Wl[l][:],
                                             start=True, stop=False)
                            nc.tensor.matmul(
                                x1p[:], lhsT=ohgT[:, t2 * P:(t2 + 1) * P],
                                rhs=x2rm[:, gc, :], start=False, stop=True)
                            nc.scalar.activation(comb[:, j, :], x1p[:],
                                                 AF.Copy)
                        # elu(x) = exp(min(x,0)) - 1 + relu(x)
                        cf = comb[:, :sl, :].rearrange("p a b -> p (a b)")
                        F = sl * C
                        u = mp.tile([P, cfg.SWATH * C], f32, tag="neg")
                        nc.scalar.activation(u[:, :F], cf, AF.Relu,
                                             scale=-1.0)      # -min(x,0)
                        nc.scalar.activation(u[:, :F], u[:, :F], AF.Exp,
                                             scale=-1.0)      # exp(min(x,0))
                        r = mp.tile([P, cfg.SWATH * C], f32, tag="ex")
                        nc.scalar.activation(r[:, :F], cf, AF.Relu)
                        ho = hs_out[gc][:, s0:s0 + sl, :]
                        nc.vector.scalar_tensor_tensor(
                            ho.rearrange("p a b -> p (a b)"), u[:, :F], -1.0,
                            r[:, :F], op0=OP.add, op1=OP.add)
                hs_in, hs_out = hs_out, hs_in

            # ---- head ------------------------------------------------------
            hgT = mp.tile([P, cfg.NGC * P], bf16, tag="hgT")
            for gc in range(cfg.NGC):
                psg = pacc.tile([P, D], f32, tag="acc")
                for t2 in range(cfg.T2):
                    nc.tensor.matmul(
                        psg[:], lhsT=ohg[gc][:, t2, :],
                        rhs=hs_in[gc][:, t2, :],
                        start=(t2 == 0), stop=(t2 == cfg.T2 - 1))
                gm = mp.tile([P, D], bf16, tag="gm")
                nc.scalar.activation(gm[:], psg[:], AF.Copy,
                                     scale=invg[:, gc:gc + 1])
                ptt = ptr.tile([P, P], bf16, tag="tr")
                nc.tensor.transpose(ptt[:], gm[:], ident[:])
                nc.scalar.activation(hgT[:, gc * P:(gc + 1) * P], ptt[:], AF.Copy)

            y1 = []
            for h in range(2):
                yps = pwide.tile([P, cfg.NGC * P], f32, tag="wide")
                nc.tensor.matmul(yps[:], lhsT=Wf1[:, h * C:(h + 1) * C],
                                 rhs=hgT[:], start=True, stop=True)
                y1t = mp.tile([P, cfg.NGC * P], bf16, tag=f"y1_{h}")
                nc.scalar.activation(y1t[:], yps[:], AF.Relu,
                                     bias=bf1[:, h:h + 1])
                y1.append(y1t)
            y2ps = pwide.tile([P, cfg.NGC * P], f32, tag="wide")
            for h in range(2):
                nc.tensor.matmul(y2ps[:10, :], lhsT=Wf2[:, h, :],
                                 rhs=y1[h][:], start=(h == 0), stop=(h == 1))
            yout = mp.tile([P, cfg.NGC * P], f32, tag="yout")
            nc.scalar.activation(yout[:10, :], y2ps[:10, :], AF.Identity,
                                 bias=bf2_t[:10, :])
            nc.sync.dma_start(out_d[:], yout[:10, :])

    nc.compile()
    return nc


# ---------------------------------------------------------------------------
# entry point
# ---------------------------------------------------------------------------

_CACHED = {}


def _get_nc(cfg):
    key = (cfg.W, cfg.T1, cfg.T2, cfg.NGC, cfg.G_SH, cfg.NCORES, cfg.SWATH)
    if key not in _CACHED:
        _CACHED[key] = build_bass(cfg)
    return _CACHED[key]


def make_in_maps(cfg, inputs):
    plans = plan(cfg, inputs["h_node"], inputs["subgraph_batch"],
                 inputs["subgraph_idx_batch"])
    iota = np.broadcast_to(
        np.arange(P, dtype=np.float32), (P, P)).astype(BF16)
    iotag = np.broadcast_to(
        np.arange(P, dtype=np.float32)[:, None], (P, cfg.NGC)).copy()
    ident = np.eye(P, dtype=BF16)
    shared = {
        "iota": iota,
        "iotag": iotag,
        "ident": ident,
        "W0": np.asarray(inputs["W_fc0"], np.float32),
        "Ws0": np.asarray(inputs["W_sum0"], np.float32),
        "b0": np.asarray(inputs["b_fc0"], np.float32),
        "bs0": np.asarray(inputs["b_sum0"], np.float32),
        "W1": np.asarray(inputs["W_fc1"], np.float32),
        "Ws1": np.asarray(inputs["W_sum1"], np.float32),
        "b1": np.asarray(inputs["b_fc1"], np.float32),
        "bs1": np.asarray(inputs["b_sum1"], np.float32),
        "Wf1": np.asarray(inputs["Wf1"], np.float32),
        "bf1": np.asarray(inputs["bf1"], np.float32),
        "Wf2": np.asarray(inputs["Wf2"], np.float32),
        "bf2": np.asarray(inputs["bf2"], np.float32),
    }
    return [dict(shared, **p) for p in plans]


def run(cfg, inputs, trace=False):
    from concourse.bass_utils import run_bass_kernel_spmd

    in_maps = make_in_maps(cfg, inputs)
    nc = _get_nc(cfg)
    res = run_bass_kernel_spmd(nc, in_maps, list(range(cfg.NCORES)),
                               trace=trace)
    outs = [np.asarray(res.results[c]["out"]).T for c in range(cfg.NCORES)]
    out = np.concatenate(outs, axis=0).astype(np.float32)
    return out, res


def kernel(**inputs) -> np.ndarray:
    out, _ = run(FULL, inputs)
    return out



# revision 4
# speedup vs baseline: 1.0544x; 1.0544x over previous
"""Trainium2 Bass kernel for nn_DSnetwork (gnn_message_passing), v3.

Reference computation (S=131072 subgraphs, G=4096 graphs, N=2M nodes, D=128):
  h_sub  = segment_mean(h_node, subgraph_batch, S)            # [S,128]
  2x DS layers:
    x1 = h_sub @ W + b
    x2 = segment_mean(h_sub, subgraph_idx_batch, G) @ Ws + bs
    h_sub = elu(x1 + x2[subgraph_idx_batch])
  h_graph = segment_mean(h_sub, subgraph_idx_batch, G)
  out = relu(h_graph @ Wf1 + bf1) @ Wf2 + bf2                 # [G,10]

v3 changes vs baseline:
  * 2:1 same-segment pair reduction done by the DMA engines (CCE accumulate
    on the SWDGE path): host packs node pairs so the "odd" halves land on
    top of the "even" halves with accum_op=add.  Halves phase-1 matmul and
    one-hot-build work at zero engine cost.
  * 4-window grouped DMAs (>=1MB transfers) for HBM bandwidth.
  * Per-graph-chunk pipelining: DS layers of chunk g overlap phase-1 DMA
    and matmuls of chunk g+1.
  * ELU computed straight from PSUM (min/max on VectorE, exp on ScalarE),
    no intermediate copies.
  * Graph one-hots built once per chunk, reused across both layers.
"""

from dataclasses import dataclass

import ml_dtypes
import numpy as np

BF16 = ml_dtypes.bfloat16
P = 128


@dataclass(frozen=True)
class Cfg:
    D: int = 128          # node feature dim
    C: int = 128          # hidden dim
    NCORES: int = 8
    G_SH: int = 512       # graphs per core
    NGC: int = 4          # graph chunks of 128 graphs per core
    T2: int = 34          # seg tiles (=windows of 128 segs) per graph chunk
    NSLOT: int = 9        # matmul slots per window (8 pair + 1 single)
    NSHIP: int = 17       # shipped tiles per window (9 base + 8 odd)
    WG: int = 4           # windows per DMA group
    SWATH: int = 4        # seg tiles per elu swath

    @property
    def NS(self):         # windows (seg tiles) per core
        return self.NGC * self.T2

    @property
    def SMAXP(self):      # padded segs per core
        return self.NS * P

    @property
    def NGRP(self):       # DMA groups per graph chunk (ragged tail)
        return (self.T2 + self.WG - 1) // self.WG


FULL = Cfg()

# ---------------------------------------------------------------------------
# host-side planner: shard + pair packing + metadata
# ---------------------------------------------------------------------------


def _plan_core(cfg, core, h_node_bf16, sb, sib, seg_cnt, g_cnt):
    g0 = core * cfg.G_SH
    D = cfg.D

    ngrp_tot = cfg.NGC * cfg.NGRP
    # shipped layout per group: [P, WG*9 base tiles | WG*8 odd tiles] * D
    hp = np.zeros((ngrp_tot, P, cfg.WG * cfg.NSHIP * D), dtype=BF16)
    rel = np.full((P, cfg.NS * cfg.NSLOT, 2), -1.0, dtype=BF16)
    invs = np.zeros((P, cfg.NS), dtype=np.float32)
    rel2 = np.full((P, cfg.NS, 2), -1.0, dtype=BF16)
    invg = np.zeros((P, cfg.NGC), dtype=np.float32)
    gid_pad = np.full(cfg.SMAXP, -1.0, dtype=np.float64)

    for gc in range(cfg.NGC):
        glo = g0 + gc * P
        ghi = glo + P
        a = int(np.searchsorted(sib, glo))
        b = int(np.searchsorted(sib, ghi))
        nseg = b - a
        assert nseg <= cfg.T2 * P, f"T2 too small: {nseg} > {cfg.T2 * P}"
        base_tile = gc * cfg.T2

        gl = (sib[a:b] - glo).astype(np.int64)
        gid_pad[base_tile * P:base_tile * P + nseg] = gl

        r2 = np.full(cfg.T2 * P, -1.0, dtype=BF16)
        r2[:nseg] = gl.astype(BF16)
        rel2[:, base_tile:base_tile + cfg.T2, :] = \
            r2.reshape(cfg.T2, P).T[:, :, None]

        ivs = np.zeros(cfg.T2 * P, dtype=np.float32)
        ivs[:nseg] = 1.0 / np.maximum(seg_cnt[a:b], 1).astype(np.float32)
        invs[:, base_tile:base_tile + cfg.T2] = ivs.reshape(cfg.T2, P).T

        invg[:, gc] = 1.0 / np.maximum(g_cnt[glo:ghi], 1).astype(np.float32)

        seg_starts = np.searchsorted(sb, np.arange(a, b + 1))
        for w in range(cfg.T2):
            slo = w * P
            shi = min(slo + P, nseg)
            if slo >= nseg:
                continue
            k = base_tile + w                     # global window index
            grp = gc * cfg.NGRP + w // cfg.WG     # DMA group
            wi = w % cfg.WG                       # window within group
            nlo = int(seg_starts[slo])
            nhi = int(seg_starts[shi])
            if nhi == nlo:
                continue
            segs_w = np.arange(slo, shi)
            cnts = seg_starts[segs_w + 1] - seg_starts[segs_w]   # per-seg node count
            n_idx = np.arange(nlo, nhi)
            segl = (sb[nlo:nhi] - (a + slo)).astype(np.int64)     # rel seg in [0,128)
            rank = n_idx - seg_starts[segl + slo]                 # rank within seg
            c_of_n = cnts[segl]
            is_single = (rank == c_of_n - 1) & (c_of_n % 2 == 1)
            pair_id_in_seg = rank // 2
            pairs_per_seg = cnts // 2
            pair_base = np.concatenate(([0], np.cumsum(pairs_per_seg)[:-1]))
            pair_id = pair_base[segl] + pair_id_in_seg            # for non-singles
            n_pairs = int(pairs_per_seg.sum())
            n_sing = int(is_single.sum())
            assert n_pairs <= 8 * P, f"pair overflow {n_pairs} @ core{core} w{k}"
            assert n_sing <= P, f"single overflow {n_sing} @ core{core} w{k}"

            base0 = wi * cfg.NSLOT                 # base tiles of window in group
            odd0 = cfg.WG * cfg.NSLOT + wi * 8     # odd tiles of window in group

            npair_mask = ~is_single
            pid = pair_id[npair_mask]
            slot = pid // P
            part = pid % P
            even = rank[npair_mask] % 2 == 0
            tloc = np.where(even, base0 + slot, odd0 + slot)

            sid = np.cumsum(is_single) - 1
            spart = sid[is_single]

            flat = hp[grp].reshape(P, cfg.WG * cfg.NSHIP, D)
            flat[part, tloc] = h_node_bf16[n_idx[npair_mask]]
            flat[spart, base0 + 8] = h_node_bf16[n_idx[is_single]]

            # rel metadata (value = rel seg id, same for both pair halves)
            rel_f = rel[:, k * cfg.NSLOT:(k + 1) * cfg.NSLOT, :]
            ev = even
            rel_f[part[ev], slot[ev], :] = segl[npair_mask][ev].astype(BF16)[:, None]
            rel_f[spart, 8, :] = segl[is_single].astype(BF16)[:, None]

    gidb = np.broadcast_to(gid_pad.astype(BF16), (P, cfg.SMAXP)).copy()
    return {"hp": hp, "rel": rel, "invs": invs, "rel2": rel2,
            "invg": invg, "gidb": gidb}


def plan(cfg, h_node, sb, sib):
    sb = np.asarray(sb).astype(np.int64)
    sib = np.asarray(sib).astype(np.int64)
    S = sib.shape[0]
    G = cfg.NCORES * cfg.G_SH
    seg_cnt = np.bincount(sb, minlength=S)
    g_cnt = np.bincount(sib, minlength=G)
    h_bf16 = np.asarray(h_node).astype(BF16)
    return [
        _plan_core(cfg, c, h_bf16, sb, sib, seg_cnt, g_cnt)
        for c in range(cfg.NCORES)
    ]


# ---------------------------------------------------------------------------
# bass program
# ---------------------------------------------------------------------------


def build_bass(cfg):
    import concourse.mybir as mybir
    import concourse.tile as tile
    from concourse import bacc

    f32 = mybir.dt.float32
    bf16 = mybir.dt.bfloat16
    AF = mybir.ActivationFunctionType
    OP = mybir.AluOpType
    D, C = cfg.D, cfg.C
    TPW = cfg.T2 * P

    nc = bacc.Bacc("TRN2", target_bir_lowering=False, debug=False)

    def din(name, shape, dt=f32):
        return nc.dram_tensor(name, shape, dt, kind="ExternalInput").ap()

    hp_d = din("hp", [cfg.NGC * cfg.NGRP, P, cfg.WG * cfg.NSHIP * D], bf16)
    rel_d = din("rel", [P, cfg.NS * cfg.NSLOT, 2], bf16)
    invs_d = din("invs", [P, cfg.NS])
    rel2_d = din("rel2", [P, cfg.NS, 2], bf16)
    invg_d = din("invg", [P, cfg.NGC])
    gidb_d = din("gidb", [P, cfg.SMAXP], bf16)
    iota_d = din("iota", [P, P], bf16)
    iotaf_d = din("iotaf", [P, cfg.NSLOT * P], bf16)
    iotag_d = din("iotag", [P, cfg.NGC])
    ident_d = din("ident", [P, P], bf16)

    w_d = {}
    for l in range(2):
        w_d[f"W{l}"] = din(f"W{l}", [D, C])
        w_d[f"Ws{l}"] = din(f"Ws{l}", [D, C])
        w_d[f"b{l}"] = din(f"b{l}", [C])
        w_d[f"bs{l}"] = din(f"bs{l}", [C])
    w_d["Wf1"] = din("Wf1", [C, 2 * C])
    w_d["bf1"] = din("bf1", [2 * C])
    w_d["Wf2"] = din("Wf2", [2 * C, 10])
    w_d["bf2"] = din("bf2", [10])

    out_d = nc.dram_tensor("out", [10, cfg.G_SH], f32, kind="ExternalOutput").ap()

    with tile.TileContext(nc) as tc:
        with (
            tc.tile_pool(name="persist", bufs=1) as pp,
            tc.tile_pool(name="stream", bufs=3) as sp,
            tc.tile_pool(name="small", bufs=2) as mp,
            tc.tile_pool(name="gcbuf", bufs=2) as gp,
            tc.tile_pool(name="psum_acc", bufs=3, space="PSUM") as pacc,
            tc.tile_pool(name="psum_tr", bufs=2, space="PSUM") as ptr,
            tc.tile_pool(name="psum_sw", bufs=2, space="PSUM") as psw,
            tc.tile_pool(name="psum_wide", bufs=1, space="PSUM") as pwide,
        ):
            # ---- constants / weights ----------------------------------------
            def load(ap_dram, shape, dt):
                t = pp.tile(shape, dt, tag=f"ld_{ap_dram.tensor.name}")
                nc.sync.dma_start(t[:], ap_dram)
                return t

            iota = load(iota_d, [P, P], bf16)
            iotaf = load(iotaf_d, [P, cfg.NSLOT, P], bf16)
            iotag = load(iotag_d, [P, cfg.NGC], f32)
            ident = load(ident_d, [P, P], bf16)
            rel = load(rel_d, [P, cfg.NS * cfg.NSLOT, 2], bf16)
            invs = load(invs_d, [P, cfg.NS], f32)
            rel2 = load(rel2_d, [P, cfg.NS, 2], bf16)
            invg = load(invg_d, [P, cfg.NGC], f32)

            def cast_bf16(name, tf, shape):
                tb = pp.tile(shape, bf16, tag=f"bf_{name}")
                nc.vector.tensor_copy(tb[:], tf[:])
                return tb

            Wl, Ws, bsum = [], [], []
            for l in range(2):
                Wl.append(cast_bf16(
                    f"W{l}", load(w_d[f"W{l}"], [D, C], f32), [D, C]))
                Ws.append(cast_bf16(
                    f"Ws{l}", load(w_d[f"Ws{l}"], [D, C], f32), [D, C]))
                b_t = load(w_d[f"b{l}"].unsqueeze(1), [P, 1], f32)
                bs_t = load(w_d[f"bs{l}"].unsqueeze(1), [P, 1], f32)
                s = pp.tile([P, 1], f32, tag=f"bsum{l}")
                nc.vector.tensor_tensor(s[:], b_t[:], bs_t[:], op=OP.add)
                bsum.append(s)
            Wf1 = cast_bf16("Wf1", load(w_d["Wf1"], [C, 2 * C], f32),
                            [C, 2 * C])
            Wf2 = cast_bf16(
                "Wf2",
                load(w_d["Wf2"].rearrange("(h p) t -> p h t", h=2),
                     [P, 2, 10], f32),
                [P, 2, 10])
            bf1 = load(w_d["bf1"].rearrange("(h p) -> p h", h=2), [P, 2], f32)
            bf2_t = pp.tile([P, 1], f32, tag="ld_bf2")
            nc.sync.dma_start(bf2_t[:10, :], w_d["bf2"].unsqueeze(1))

            hgT = pp.tile([P, cfg.NGC * P], bf16, tag="hgT", name="hgT")

            # ---- per graph-chunk pipeline -----------------------------------
            for gc in range(cfg.NGC):
                # phase 1: node -> subgraph (pair-reduced) --------------------
                hs0 = gp.tile([P, cfg.T2, D], bf16, tag="hs0", name=f"hs0_{gc}")
                for grp in range(cfg.NGRP):
                    w0 = grp * cfg.WG
                    gs = min(cfg.WG, cfg.T2 - w0)
                    gidx = gc * cfg.NGRP + grp
                    pb = sp.tile([P, cfg.WG * cfg.NSLOT, D], bf16, tag="pb")
                    nb = gs * cfg.NSLOT * D
                    # base: 9 tiles per window (8 even pair halves + singles)
                    nc.sync.dma_start(
                        pb[:, :gs * cfg.NSLOT, :].rearrange("p a b -> p (a b)"),
                        hp_d[gidx, :, 0:nb])
                    # odd pair halves accumulate onto slots 0..7 of each window
                    nc.gpsimd.dma_start(
                        pb[:].rearrange("p (w s) b -> p w s b", s=cfg.NSLOT)
                             [:, 0:gs, 0:8, :],
                        hp_d[gidx, :, cfg.WG * cfg.NSLOT * D:
                             cfg.WG * cfg.NSLOT * D + gs * 8 * D]
                            .rearrange("p (w s b) -> p w s b", s=8, b=D),
                        accum_op=OP.add)
                    for wi in range(gs):
                        w = w0 + wi
                        k = gc * cfg.T2 + w
                        oh = sp.tile([P, cfg.NSLOT, P], bf16, tag="oh")
                        rb = rel[:, k * cfg.NSLOT:(k + 1) * cfg.NSLOT, :] \
                            .unsqueeze(2).to_broadcast([P, cfg.NSLOT, P // 2, 2])
                        iob = iotaf[:].rearrange("p a (b c) -> p a b c", c=2)
                        ohv = oh[:].rearrange("p a (b c) -> p a b c", c=2)
                        nc.vector.tensor_tensor(ohv, rb, iob, op=OP.is_equal)
                        ps = pacc.tile([P, D], f32, tag="acc")
                        for j in range(cfg.NSLOT):
                            nc.tensor.matmul(
                                ps[:], lhsT=oh[:, j, :],
                                rhs=pb[:, wi * cfg.NSLOT + j, :],
                                start=(j == 0), stop=(j == cfg.NSLOT - 1))
                        nc.scalar.activation(
                            hs0[:, w, :], ps[:], AF.Copy,
                            scale=invs[:, k:k + 1])

                # graph one-hots for this chunk (reused across layers) --------
                ohg = gp.tile([P, cfg.T2, P], bf16, tag="ohg", name=f"ohg_{gc}")
                r2b = rel2[:, gc * cfg.T2:(gc + 1) * cfg.T2, :] \
                    .unsqueeze(2).to_broadcast([P, cfg.T2, P // 2, 2])
                iob = iota[:].rearrange("p (a b) -> p a b", b=2) \
                    .unsqueeze(1).to_broadcast([P, cfg.T2, P // 2, 2])
                nc.vector.tensor_tensor(
                    ohg[:].rearrange("p a (b c) -> p a b c", c=2),
                    r2b, iob, op=OP.is_equal)
                ohgT = gp.tile([P, TPW], bf16, tag="ohgT", name=f"ohgT_{gc}")
                nc.sync.dma_start(ohgT[:], gidb_d[:, gc * TPW:(gc + 1) * TPW])
                nc.vector.tensor_scalar(
                    ohgT[:], ohgT[:], iotag[:, gc:gc + 1], None,
                    op0=OP.is_equal)

                # DS layers ---------------------------------------------------
                hs_in = hs0
                for l in range(2):
                    # graph mean of hs_in -> x2 (row-major)
                    psg = pacc.tile([P, D], f32, tag="acc")
                    for t2 in range(cfg.T2):
                        nc.tensor.matmul(
                            psg[:], lhsT=ohg[:, t2, :], rhs=hs_in[:, t2, :],
                            start=(t2 == 0), stop=(t2 == cfg.T2 - 1))
                    gm = mp.tile([P, D], bf16, tag="gm")
                    nc.scalar.activation(gm[:], psg[:], AF.Copy,
                                         scale=invg[:, gc:gc + 1])
                    ptt = ptr.tile([P, P], bf16, tag="tr")
                    nc.tensor.transpose(ptt[:], gm[:], ident[:])
                    gmT = mp.tile([P, P], bf16, tag="gmT")
                    nc.vector.tensor_copy(gmT[:], ptt[:])
                    x2q = pacc.tile([P, P], f32, tag="acc")
                    nc.tensor.matmul(x2q[:], lhsT=Ws[l][:], rhs=gmT[:],
                                     start=True, stop=True)
                    x2T = mp.tile([P, P], bf16, tag="x2T")
                    nc.scalar.activation(x2T[:], x2q[:], AF.Identity,
                                         bias=bsum[l][:])
                    ptt2 = ptr.tile([P, P], bf16, tag="tr")
                    nc.tensor.transpose(ptt2[:], x2T[:], ident[:])
                    x2rm = mp.tile([P, C], bf16, tag="x2rm")
                    nc.vector.tensor_copy(x2rm[:], ptt2[:])

                    hs_out = gp.tile([P, cfg.T2, D], bf16, tag=f"hs{l + 1}",
                                     name=f"hs{l + 1}_{gc}")
                    for s0 in range(0, cfg.T2, cfg.SWATH):
                        sl = min(cfg.SWATH, cfg.T2 - s0)
                        xp = psw.tile([P, cfg.SWATH, C], f32, tag="sw")
                        for j in range(sl):
                            t2 = s0 + j
                            ptt = ptr.tile([P, P], bf16, tag="tr")
                            nc.tensor.transpose(ptt[:], hs_in[:, t2, :],
                                                ident[:])
                            hT = mp.tile([P, P], bf16, tag="hT", bufs=3)
                            nc.scalar.activation(hT[:], ptt[:], AF.Copy)
                            nc.tensor.matmul(xp[:, j, :], lhsT=hT[:],
                                             rhs=Wl[l][:],
                                             start=True, stop=False)
                            nc.tensor.matmul(
                                xp[:, j, :],
                                lhsT=ohgT[:, t2 * P:(t2 + 1) * P],
                                rhs=x2rm[:], start=False, stop=True)
                        # elu(x) = exp(min(x,0)) - 1 + max(x,0), from PSUM
                        xf = xp[:, :sl, :].rearrange("p a b -> p (a b)")
                        F = sl * C
                        em = mp.tile([P, cfg.SWATH * C], bf16, tag="em")
                        nc.vector.tensor_scalar(em[:, :F], xf, 0.0, None,
                                                op0=OP.min)
                        rm = mp.tile([P, cfg.SWATH * C], bf16, tag="rm")
                        nc.vector.tensor_scalar(rm[:, :F], xf, 0.0, None,
                                                op0=OP.max)
                        nc.scalar.activation(em[:, :F], em[:, :F], AF.Exp)
                        nc.vector.scalar_tensor_tensor(
                            hs_out[:, s0:s0 + sl, :]
                                .rearrange("p a b -> p (a b)"),
                            em[:, :F], -1.0, rm[:, :F],
                            op0=OP.add, op1=OP.add)
                    hs_in = hs_out

                # head graph mean for this chunk ------------------------------
                psg = pacc.tile([P, D], f32, tag="acc")
                for t2 in range(cfg.T2):
                    nc.tensor.matmul(
                        psg[:], lhsT=ohg[:, t2, :], rhs=hs_in[:, t2, :],
                        start=(t2 == 0), stop=(t2 == cfg.T2 - 1))
                gm = mp.tile([P, D], bf16, tag="gm")
                nc.scalar.activation(gm[:], psg[:], AF.Copy,
                                     scale=invg[:, gc:gc + 1])
                ptt = ptr.tile([P, P], bf16, tag="tr")
                nc.tensor.transpose(ptt[:], gm[:], ident[:])
                nc.vector.tensor_copy(hgT[:, gc * P:(gc + 1) * P], ptt[:])

            # ---- head -------------------------------------------------------
            y1 = []
            for h in range(2):
                yps = pwide.tile([P, cfg.NGC * P], f32, tag="wide")
                nc.tensor.matmul(yps[:], lhsT=Wf1[:, h * C:(h + 1) * C],
                                 rhs=hgT[:], start=True, stop=True)
                y1t = mp.tile([P, cfg.NGC * P], bf16, tag=f"y1_{h}")
                nc.scalar.activation(y1t[:], yps[:], AF.Relu,
                                     bias=bf1[:, h:h + 1])
                y1.append(y1t)
            y2ps = pwide.tile([P, cfg.NGC * P], f32, tag="wide")
            for h in range(2):
                nc.tensor.matmul(y2ps[:10, :], lhsT=Wf2[:, h, :],
                                 rhs=y1[h][:], start=(h == 0), stop=(h == 1))
            yout = mp.tile([P, cfg.NGC * P], f32, tag="yout")
            nc.scalar.activation(yout[:10, :], y2ps[:10, :], AF.Identity,
                                 bias=bf2_t[:10, :])
            nc.sync.dma_start(out_d[:], yout[:10, :])

    nc.compile()
    return nc


# ---------------------------------------------------------------------------
# entry point
# ---------------------------------------------------------------------------

_CACHED = {}


def _get_nc(cfg):
    key = (cfg.NSLOT, cfg.T2, cfg.NGC, cfg.G_SH, cfg.NCORES, cfg.SWATH,
           cfg.WG)
    if key not in _CACHED:
        _CACHED[key] = build_bass(cfg)
    return _CACHED[key]


def make_in_maps(cfg, inputs):
    plans = plan(cfg, inputs["h_node"], inputs["subgraph_batch"],
                 inputs["subgraph_idx_batch"])
    iota = np.broadcast_to(
        np.arange(P, dtype=np.float32), (P, P)).astype(BF16)
    iotaf = np.broadcast_to(
        np.arange(P, dtype=np.float32), (P, cfg.NSLOT, P)) \
        .reshape(P, cfg.NSLOT * P).astype(BF16)
    iotag = np.broadcast_to(
        np.arange(P, dtype=np.float32)[:, None], (P, cfg.NGC)).copy()
    ident = np.eye(P, dtype=BF16)
    shared = {
        "iota": iota,
        "iotaf": np.ascontiguousarray(iotaf),
        "iotag": iotag,
        "ident": ident,
        "W0": np.asarray(inputs["W_fc0"], np.float32),
        "Ws0": np.asarray(inputs["W_sum0"], np.float32),
        "b0": np.asarray(inputs["b_fc0"], np.float32),
        "bs0": np.asarray(inputs["b_sum0"], np.float32),
        "W1": np.asarray(inputs["W_fc1"], np.float32),
        "Ws1": np.asarray(inputs["W_sum1"], np.float32),
        "b1": np.asarray(inputs["b_fc1"], np.float32),
        "bs1": np.asarray(inputs["b_sum1"], np.float32),
        "Wf1": np.asarray(inputs["Wf1"], np.float32),
        "bf1": np.asarray(inputs["bf1"], np.float32),
        "Wf2": np.asarray(inputs["Wf2"], np.float32),
        "bf2": np.asarray(inputs["bf2"], np.float32),
    }
    return [dict(shared, **p) for p in plans]


def run(cfg, inputs, trace=False):
    from concourse.bass_utils import run_bass_kernel_spmd

    in_maps = make_in_maps(cfg, inputs)
    nc = _get_nc(cfg)
    res = run_bass_kernel_spmd(nc, in_maps, list(range(cfg.NCORES)),
                               trace=trace)
    outs = [np.asarray(res.results[c]["out"]).T for c in range(cfg.NCORES)]
    out = np.concatenate(outs, axis=0).astype(np.float32)
    return out, res


def kernel(**inputs) -> np.ndarray:
    out, _ = run(FULL, inputs)
    return out
